# revision 20
# baseline (speedup 1.0000x reference)
"""BERT self-attention (B=4, S=1024, HID=1024, NH=16, HD=64) on 8 TRN2 NeuronCores.

Sharding: 8 shards = 4 batches x 2 head-halves. Core c handles batch c%4 and
heads [g*8, g*8+8) with g = c//4. Each core computes q/k/v projections for its
512 feature columns and full attention for its 8 heads; no collectives needed.
The host pre-transposes hidden_states / weights so the device never transposes.

Device-side layout choices:
  - q^T, k^T kept as [feat, seq] (feat on partitions): scores are computed
    TRANSPOSED, s^T[keys, queries] = k^T.T @ q^T, so softmax's exp needs no
    reduction first and the attention-mask bias is a per-partition ACT bias.
  - exp(s/8 + maskbias) goes straight from PSUM through the scalar engine into
    SBUF as unnormalized probabilities p~^T [keys, queries]; two key-chunks are
    paired per ACT op (N=1024) to amortize the ~352-cycle ACT fixed cost.
  - v is produced as [seq, feat] with a ones-column appended per head
    (v_aug [seq, 65]); ctx~^T = v_aug.T @ p~^T and row 64 of the PSUM result is
    the softmax denominator. Normalize: DVE reciprocal (straight off PSUM) +
    gpsimd partition broadcast + DVE multiply. v's bias is added by the DVE
    during the PSUM->SBUF copy (bv pre-broadcast across partitions once).
  - all matmuls are fp16 data (fp8 was measured at ~2e-2 rel err - too lossy).
  - emission interleaves q/k projection chunks into the attention loop so the
    PE stays busy (and HAM-warm) while the scalar engine chews through exp.

Schedule/bandwidth choices:
  - inputs stream over FOUR DGE rings (sync/vector: hidden states per
    contraction chunk; scalar/gpsimd: weights per feature chunk, fc-major
    host packing) ordered so the first qk0 matmuls' tiles land first.
  - ~24 throwaway matmuls on a zeroed tile run during the DMA prologue to
    bring the PE out of its low p-state before the first real matmul.
  - the last iteration is split into two 256-query chunks so the final
    exp->ctx->normalize->store chain after the last big matmul is short.
Host reassembles: out[h] is ctx^T [64, 1024] fp16 -> transpose -> out columns.
"""
import os
import sys
from contextlib import ExitStack

for _p in ("/root/.axon_site/_ro/trn_rl_repo", "/opt/trn_rl_repo"):
    if os.path.isdir(_p) and _p not in sys.path:
        sys.path.append(_p)

import numpy as np
import concourse.bacc as bacc
import concourse.mybir as mybir
from concourse import tile
from concourse.bass_utils import run_bass_kernel_spmd

B, S, HID, NH, HD = 4, 1024, 1024, 16, 64
NCORES = 8
FSH = 512  # feature columns per core = 8 heads * 64
HC = 8  # hid contraction chunks of 128
JC = 8  # key/seq chunks of 128
FC = 4  # feature chunks of 128
NHL = 8  # local heads per core
NWARM = 24  # p-state warmup matmuls during the DMA prologue

# feature flags (bisectable): rebuild with K._NC=None after changing
WARMUP = True  # PE p-state warmup matmuls during DMA prologue
TAILSPLIT = True  # split last 512-query chunk into 2x256 for a short tail
DMANEW = True  # fc-major weight packing + fine-grained multi-ring DMA order
OUT16 = True  # fp16 output store (host casts back to fp32)

F32 = mybir.dt.float32
F32R = mybir.dt.float32r
F16 = mybir.dt.float16
EXP = mybir.ActivationFunctionType.Exp


def _r(ap):
    return ap.bitcast(F32R)


def _build_nc():
    nc = bacc.Bacc(None, target_bir_lowering=False, debug=False)

    wshape = [128, FC, HC, 128] if DMANEW else [128, HC, FSH]
    hsT = nc.declare_dram_parameter("hsT", [128, HC, S], F16, isOutput=False)
    wqT = nc.declare_dram_parameter("wqT", wshape, F16, isOutput=False)
    wkT = nc.declare_dram_parameter("wkT", wshape, F16, isOutput=False)
    wvT = nc.declare_dram_parameter("wvT", [128, HC, FSH], F16, isOutput=False)
    bqc = nc.declare_dram_parameter("bqc", [128, FC], F32, isOutput=False)
    bkc = nc.declare_dram_parameter("bkc", [128, FC], F32, isOutput=False)
    bv1 = nc.declare_dram_parameter("bv1", [1, FSH], F32, isOutput=False)
    mb = nc.declare_dram_parameter("mb", [128, JC], F32, isOutput=False)
    out = nc.declare_dram_parameter(
        "out", [NHL, HD, S], F16 if OUT16 else F32, isOutput=True
    )

    with tile.TileContext(nc) as tc, ExitStack() as ctx:
        ctx.enter_context(
            nc.allow_low_precision(reason="fp16 matmuls; fp16 output store")
        )
        const = ctx.enter_context(tc.tile_pool(name="const", bufs=1))
        ps_p = ctx.enter_context(tc.tile_pool(name="ps_p", bufs=2, space="PSUM"))
        p_pool = ctx.enter_context(tc.tile_pool(name="p", bufs=2))
        sm = ctx.enter_context(tc.tile_pool(name="sm", bufs=2))

        hsT_sb = const.tile([128, HC, S], F16, tag="hsT")
        wq_sb = const.tile(wshape, F16, tag="wq")
        wk_sb = const.tile(wshape, F16, tag="wk")
        wv_sb = const.tile([128, HC, FSH], F16, tag="wv")
        bq_sb = const.tile([128, FC], F32, tag="bq")
        bk_sb = const.tile([128, FC], F32, tag="bk")
        bv_sb = const.tile([1, FSH], F32R, tag="bv")
        mb_sb = const.tile([128, JC], F32, tag="mb")

        def wslice(w_sb, fc, hc):
            """weight [128, fc, hc, 128] slice under either host packing."""
            if DMANEW:
                return w_sb[:, fc, hc, :]
            return w_sb[:, hc, fc * 128 : (fc + 1) * 128]

        if DMANEW:
            # two HWDGE rings drain inputs in parallel, ordered so the tiles
            # the first qk0 stages touch land first: hsT per chunk pair on
            # sync (small consts up front); weights per feature chunk
            # (fc-major host packing) on scalar, fc0 first, wv before fc2/3.
            nc.sync.dma_start(bq_sb[:], bqc[:])
            nc.sync.dma_start(bk_sb[:], bkc[:])
            nc.sync.dma_start(mb_sb[:], mb[:])
            nc.sync.dma_start(bv_sb[:], _r(bv1[:]))
            for hc in range(0, HC, 2):
                nc.sync.dma_start(
                    hsT_sb[:, hc : hc + 2, :], hsT[:, hc : hc + 2, :]
                )
            for fc in range(2):
                nc.scalar.dma_start(wq_sb[:, fc], wqT[:, fc])
                nc.scalar.dma_start(wk_sb[:, fc], wkT[:, fc])
            nc.scalar.dma_start(wv_sb[:], wvT[:])
            for fc in range(2, FC):
                nc.scalar.dma_start(wq_sb[:, fc], wqT[:, fc])
                nc.scalar.dma_start(wk_sb[:, fc], wkT[:, fc])
        else:
            HH = HC // 2
            h1, h2 = slice(0, HH), slice(HH, HC)
            nc.sync.dma_start(hsT_sb[:, h1, :], hsT[:, h1, :])
            nc.scalar.dma_start(wq_sb[:, h1, :], wqT[:, h1, :])
            nc.scalar.dma_start(wk_sb[:, h1, :], wkT[:, h1, :])
            nc.sync.dma_start(hsT_sb[:, h2, :], hsT[:, h2, :])
            nc.scalar.dma_start(wq_sb[:, h2, :], wqT[:, h2, :])
            nc.scalar.dma_start(wk_sb[:, h2, :], wkT[:, h2, :])
            nc.sync.dma_start(wv_sb[:], wvT[:])
            nc.sync.dma_start(bq_sb[:], bqc[:])
            nc.sync.dma_start(bk_sb[:], bkc[:])
            nc.sync.dma_start(bv_sb[:], _r(bv1[:]))
            nc.sync.dma_start(mb_sb[:], mb[:])

        ones_sb = const.tile([1, 128], F32R, tag="ones")
        nc.vector.memset(ones_sb[:].bitcast(F32), 1.0)

        qT_sb = const.tile([128, FC, S], F16, tag="qT")
        kT_sb = const.tile([128, FC, S], F16, tag="kT")
        # v with per-head ones column: [seq_part, jc, head, 64 v + 1 one]
        v_sb = const.tile([128, JC, NHL, HD + 1], F16, tag="v")
        nc.vector.memset(v_sb[:], 1.0)

        def emit_qk_proj(fc, which=None):
            """q^T,k^T projection for feature chunk fc (pack fc's 2 heads).
            which=0 emits only q, which=1 only k, None both."""
            parts = ((wq_sb, bq_sb, qT_sb), (wk_sb, bk_sb, kT_sb))
            if which is not None:
                parts = (parts[which],)
            for w_sb, b_sb, dst in parts:
                for sc in range(2):
                    ps = ps_p.tile([128, 512], F32, tag="pp", name=f"pp{fc}{sc}")
                    for hc in range(HC):
                        nc.tensor.matmul(
                            ps[:],
                            wslice(w_sb, fc, hc),
                            hsT_sb[:, hc, sc * 512 : (sc + 1) * 512],
                            start=(hc == 0),
                            stop=(hc == HC - 1),
                        )
                    nc.vector.tensor_scalar_add(
                        dst[:, fc, sc * 512 : (sc + 1) * 512],
                        ps[:],
                        b_sb[:, fc : fc + 1],
                    )

        def emit_v_proj():
            """v projection [seq, feat] + bias via ones-matmul."""
            for jc in range(JC):
                ps = ps_p.tile([128, 512], F32, tag="pp", name=f"ppv{jc}")
                for hc in range(HC):
                    nc.tensor.matmul(
                        ps[:],
                        hsT_sb[:, hc, jc * 128 : (jc + 1) * 128],
                        wv_sb[:, hc, :],
                        start=(hc == 0),
                        stop=False,
                    )
                nc.tensor.matmul(
                    ps[:], ones_sb[:, 0:128], bv_sb[:], start=False, stop=True
                )
                nc.vector.tensor_copy(
                    v_sb[:, jc, :, 0:HD], ps[:].rearrange("p (h d) -> p h d", h=NHL)
                )

        def emit_scores_jc(g2, qs, w, jc, ptb):
            """One key-chunk of scores + exp for pack g2, queries [qs,qs+w).
            The two heads are row-tiled on the PE (K=64 each) and share one
            ACT exp (same key-chunk -> same mask bias, exact for any mask)."""
            # each head's output starts at a fixed 512-col (2KB bank) offset:
            # matmul groups at sub-bank PSUM offsets corrupt the bank.
            ps = ps_s.tile([128, 2, 512], F32, tag="ss", name=f"ss{jc}")
            for hh in range(2):
                lo = hh * 64
                nc.tensor.matmul(
                    ps[:, hh, 0:w],
                    kT_sb[lo : lo + 64, g2, jc * 128 : (jc + 1) * 128],
                    qT_sb[lo : lo + 64, g2, qs : qs + w],
                    start=True,
                    stop=True,
                    tile_position=(lo, 0),
                )
            nc.scalar.activation(
                ptb[:, :, jc, 0:w],
                ps[:, :, 0:w],
                EXP,
                bias=mb_sb[:, jc : jc + 1],
                scale=0.125,
            )

        def ctx_psums():
            return [
                ps_c.tile([HD + 1, 512], F32, tag="cc", name=f"cc{hh}")
                for hh in (0, 1)
            ]

        def emit_ctx_jc(pcs, g2, qs, w, jc, ptb):
            for hh in range(2):
                nc.tensor.matmul(
                    pcs[hh][:, 0:w],
                    v_sb[:, jc, 2 * g2 + hh, :],
                    ptb[:, hh, jc, 0:w],
                    start=(jc == 0),
                    stop=(jc == JC - 1),
                )

        def emit_ctx_norm(pcs, g2, qs, w):
            """normalize + store both heads of pack g2, queries [qs,qs+w)."""
            for hh in range(2):
                h = 2 * g2 + hh
                pc = pcs[hh]
                den = sm.tile([1, 512], F32, tag="den", name=f"dn{hh}")
                nc.vector.tensor_copy(den[:, 0:w], pc[HD : HD + 1, 0:w])
                recip = sm.tile([1, 512], F32, tag="recip", name=f"rc{hh}")
                nc.vector.reciprocal_approx_fast(recip[:, 0:w], den[:, 0:w])
                pbs = sm.tile([64, 512], F32, tag="pbs", name=f"pb{hh}")
                nc.gpsimd.partition_broadcast(pbs[:, 0:w], recip[0:1, 0:w])
                ob = sm.tile([64, 512], F16 if OUT16 else F32, tag="ob", name=f"ob{hh}")
                nc.vector.tensor_mul(ob[:, 0:w], pc[0:HD, 0:w], pbs[:, 0:w])
                nc.sync.dma_start(out[h, :, qs : qs + w], ob[:, 0:w])

        # ---- PE p-state warmup + qk0 during the DMA prologue ----
        # The PE runs ~1.5-2x slow until it has executed ~3us continuously;
        # burn that on throwaway matmuls over a zeroed tile while the input
        # DMA is still in flight, so real matmuls start at full clock.
        if WARMUP:
            with tc.tile_pool(name="warm", bufs=1) as warm_pool, tc.tile_pool(
                name="ps_w", bufs=1, space="PSUM"
            ) as ps_w:
                wt = warm_pool.tile([128, 512], F16, tag="wt")
                nc.vector.memset(wt[:], 0.0)
                wps = ps_w.tile([128, 512], F32, tag="wps")
                for _ in range(NWARM):
                    nc.tensor.matmul(
                        wps[:], wt[:, 0:128], wt[:], start=True, stop=True
                    )
        # pack-0 q/k projections, staged per contraction-chunk pair in DMA
        # arrival order so the first matmuls only gate on the first tiles;
        # each iteration of the main loop then gets dependency-free
        # projection matmuls as PE filler while ACT chews this pack's exp:
        #   iter0: v-projection (ctx(0,0) needs it at iter end)
        #   iter1: pack-1 q+k;  iter2/3: pack-2 q/k;  iter4/5: pack-3 q/k
        with tc.tile_pool(name="ps_p0", bufs=4, space="PSUM") as ps_p0:
            qk0 = []
            for w_sb, b_sb, dst in ((wq_sb, bq_sb, qT_sb), (wk_sb, bk_sb, kT_sb)):
                for sc in range(2):
                    ps = ps_p0.tile([128, 512], F32, tag="pp0", name=f"p0{sc}")
                    qk0.append((ps, w_sb, b_sb, dst, sc))
            stages = (
                [(2 * hcp, 2 * hcp + 1) for hcp in range(4)]
                if DMANEW
                else [(0, 1, 2, 3), (4, 5, 6, 7)]
            )
            for stage in stages:
                for ps, w_sb, b_sb, dst, sc in qk0:
                    for hc in stage:
                        nc.tensor.matmul(
                            ps[:],
                            wslice(w_sb, 0, hc),
                            hsT_sb[:, hc, sc * 512 : (sc + 1) * 512],
                            start=(hc == 0),
                            stop=(hc == HC - 1),
                        )
            for ps, w_sb, b_sb, dst, sc in qk0:
                nc.vector.tensor_scalar_add(
                    dst[:, 0, sc * 512 : (sc + 1) * 512], ps[:], b_sb[:, 0:1]
                )
        ps_s = ctx.enter_context(tc.tile_pool(name="ps_s", bufs=2, space="PSUM"))
        ps_c = ctx.enter_context(tc.tile_pool(name="ps_c", bufs=2, space="PSUM"))
        fillers = [
            emit_v_proj,
            lambda: emit_qk_proj(1),
            lambda: emit_qk_proj(2, which=0),
            lambda: emit_qk_proj(2, which=1),
            lambda: emit_qk_proj(3, which=0),
            lambda: emit_qk_proj(3, which=1),
        ]
        # software pipeline one iteration deep: iteration N's score pairs are
        # interleaved with iteration N-1's ctx accumulation so ACT always has
        # a fresh scores PSUM to exp while the PE keeps streaming. The last
        # 512-query chunk is split in two so the end-of-kernel exp->ctx->
        # normalize->store chain hangs off a 256-wide (cheap) tail.
        chunks = [(g2, qs, 512) for g2 in range(4) for qs in (0, 512)]
        if TAILSPLIT:
            chunks[-1:] = [(3, 512, 256), (3, 768, 256)]
        prev = None
        for step, (g2, qs, w) in enumerate(chunks):
            ptb = p_pool.tile([128, 2, JC, 512], F16, tag="pt", name="ptb")
            for jc in range(JC):
                emit_scores_jc(g2, qs, w, jc, ptb)
                if prev is not None:
                    emit_ctx_jc(prev[0], prev[1], prev[2], prev[3], jc, prev[4])
            if step < len(fillers):
                fillers[step]()
            if prev is not None:
                emit_ctx_norm(prev[0], prev[1], prev[2], prev[3])
            prev = (ctx_psums(), g2, qs, w, ptb)
        for jc in range(JC):
            emit_ctx_jc(prev[0], prev[1], prev[2], prev[3], jc, prev[4])
        emit_ctx_norm(prev[0], prev[1], prev[2], prev[3])

    nc.compile()
    return nc


_NC = None


def _get_nc():
    global _NC
    if _NC is None:
        _NC = _build_nc()
    return _NC


# test-harness knobs (ignored in normal grading use)
TRACE = False
TRACE_DIR = None
LAST_RESULT = None


def _pack(mT):
    """[1024, N] contraction-major -> [128, 8, N] partition-major fp16 so one
    DMA moves contiguous bytes per partition (big DMA packets)."""
    n = mT.shape[1]
    return np.ascontiguousarray(
        mT.reshape(HC, 128, n).transpose(1, 0, 2)
    ).astype(np.float16)


def _pack_w(mT):
    """[1024, 512] weight -> [128, FC, HC, 128] fp16: partition-major like
    _pack but feature-chunk-major in the free dim, so one fc (what one qk
    projection stage needs) is a single contiguous DMA piece."""
    return np.ascontiguousarray(
        mT.reshape(HC, 128, FC, 128).transpose(1, 2, 0, 3)
    ).astype(np.float16)


def kernel(hidden_states, attention_mask, Wq, bq, Wk, bk, Wv, bv):
    global LAST_RESULT
    hs = np.asarray(hidden_states, dtype=np.float32)
    mask = np.asarray(attention_mask, dtype=np.float32)
    Wq = np.asarray(Wq, dtype=np.float32)
    Wk = np.asarray(Wk, dtype=np.float32)
    Wv = np.asarray(Wv, dtype=np.float32)
    bq = np.asarray(bq, dtype=np.float32)
    bk = np.asarray(bk, dtype=np.float32)
    bv = np.asarray(bv, dtype=np.float32)

    in_maps = []
    for c in range(NCORES):
        b, g = c % B, c // B
        sl = slice(g * FSH, (g + 1) * FSH)
        in_maps.append(
            {
                "hsT": _pack(hs[b].T),
                "wqT": (_pack_w if DMANEW else _pack)(Wq[sl, :].T),
                "wkT": (_pack_w if DMANEW else _pack)(Wk[sl, :].T),
                "wvT": _pack(Wv[sl, :].T),
                "bqc": np.ascontiguousarray(bq[sl].reshape(FC, 128).T),
                "bkc": np.ascontiguousarray(bk[sl].reshape(FC, 128).T),
                "bv1": np.ascontiguousarray(bv[sl].reshape(1, FSH)),
                "mb": np.ascontiguousarray(
                    ((mask[b, 0, 0, :] - 1.0) * 1.0e6).reshape(JC, 128).T
                ),
            }
        )

    nc = _get_nc()
    kw = {}
    if TRACE:
        kw = {"trace": True, "tmpdir": TRACE_DIR}
    res = run_bass_kernel_spmd(nc, in_maps, list(range(NCORES)), **kw)
    LAST_RESULT = res

    full = np.empty((B, S, HID), dtype=np.float32)
    for c in range(NCORES):
        b, g = c % B, c // B
        o = res.results[c]["out"]  # [NHL, HD, S] fp16
        full[b, :, g * FSH : (g + 1) * FSH] = (
            o.transpose(2, 0, 1).reshape(S, FSH).astype(np.float32)
        )
    return full
